# revision 71
# baseline (speedup 1.0000x reference)
"""Trainium2 Bass kernel for nn_EquiAdapter (8-core data-parallel over batch).

Math (per batch b, equi-channel n, token t):
  h_slices = h_prime.reshape(B, n_equi, T, 8, 128)
  equi_out[t,g,o] = sum_i h[t,g,i] * We[o,i]
  inv_feat = max_g h_slices                       [t, 128]
  q = inv_feat @ Wq.T + bq ; k = h_llm @ Wk.T + bk ; v = h_llm @ Wv.T + bv
  attn = softmax(q @ k.T / sqrt(128))             [t, 64]
  ctx = attn @ v ; gate = sigmoid(ctx)
  gamma = ctx @ Wg.T + bg ; beta = ctx @ Wb.T + bb
  out = h_prime + bcast_g(gate) * (bcast_g(gamma)*equi_out + bcast_g(beta))

v3: engine-balanced + ramp-optimized, targeting the ~115us DMA floor.
  - DMA issue order interleaves setup weights with the first h chunks so the
    k-projection chain (needed by the first attention) completes early.
  - Attention smalls: exp once over all 4 sub-tiles; attn weights normalized
    (one DVE op) instead of per-s ctx scaling; ctxT computed directly by PE
    (v16 stationary); P/Q built straight from gamma/beta PSUM.
  - equi -> fp32 PSUM (one 2-bank tile) -> ACT copies to bf16 -> one big DVE
    2x multiply (*P) and one Pool 2x add (+Q) in place; residual split
    DVE/Pool.  0.5 escape-scaling folded into Wg/Wb/bg/bb at setup.
"""

import sys

sys.path.insert(0, "/opt/trn_rl_repo")

import numpy as np

import concourse.bass as bass
import concourse.bacc as bacc
import concourse.tile as tile
from concourse import mybir
from concourse import masks
from concourse.bass_utils import run_bass_kernel_spmd

dt = mybir.dt
AF = mybir.ActivationFunctionType

B, N_EQUI, T, D = 8, 4, 1024, 1024
T_TEXT, D_LLM = 64, 2048
G, BK = 8, 128  # n_group, blocks
SCALE = BK ** (-0.5)
TC = 512  # token chunk
NSUB = TC // 128  # 4
NCHUNK = (N_EQUI * T) // TC  # 8 chunks per core


def _build_program():
    nc = bacc.Bacc("TRN2", target_bir_lowering=False, debug=False)

    h_dram = nc.declare_dram_parameter("h", [N_EQUI, T, D], dt.float32, isOutput=False)
    hl_dram = nc.declare_dram_parameter("hl", [T_TEXT, D_LLM], dt.float32, isOutput=False)
    w_dram = {}
    for name, shape in [
        ("Wq", [BK, BK]), ("Wk", [BK, D_LLM]), ("Wv", [BK, D_LLM]),
        ("Wg", [BK, BK]), ("Wb", [BK, BK]), ("We", [BK, BK]),
        ("bq", [BK]), ("bk", [BK]), ("bv", [BK]), ("bg", [BK]), ("bb", [BK]),
    ]:
        w_dram[name] = nc.declare_dram_parameter(name, shape, dt.float32, isOutput=False)
    out_dram = nc.declare_dram_parameter("out", [N_EQUI, T, D], dt.float32, isOutput=True)

    import contextlib
    with tile.TileContext(nc) as tc:
        with contextlib.ExitStack() as es:
            statics = es.enter_context(tc.tile_pool(name="static", bufs=1))
            setup_pool = es.enter_context(tc.tile_pool(name="setup_f32", bufs=1))
            # PSUM: htps 2 banks + eq 2 banks + attn 4 banks = 8
            htps_ps = es.enter_context(tc.tile_pool(name="htps_ps", bufs=2, space="PSUM"))
            equi_ps = es.enter_context(tc.tile_pool(name="equi_ps", bufs=1, space="PSUM"))
            attn_ps = es.enter_context(tc.tile_pool(name="attn_ps", bufs=4, space="PSUM"))

            hin_pool = es.enter_context(tc.tile_pool(name="hin", bufs=5))
            ht_pool = es.enter_context(tc.tile_pool(name="ht16", bufs=3))
            tree_pool = es.enter_context(tc.tile_pool(name="tree", bufs=1))
            inv_pool = es.enter_context(tc.tile_pool(name="invf", bufs=2))
            eqb_pool = es.enter_context(tc.tile_pool(name="eqb", bufs=3))
            ssb_pool = es.enter_context(tc.tile_pool(name="small_sb", bufs=2))

            ident16 = statics.tile([128, 128], dt.bfloat16)
            masks.make_identity(nc, ident16[:])
            ident32 = statics.tile([128, 128], dt.float32)
            masks.make_identity(nc, ident32[:])
            ones16 = statics.tile([1, 128], dt.bfloat16)
            nc.gpsimd.memset(ones16[:], 1.0)

            setup_out = {}
            NCH = D_LLM // 128  # 16 contraction chunks

            # ---- setup DMA part A: everything the attention k-chain and the
            # equi/q paths need (small weights + h_llm + Wk)
            def setup_dma_a():
                d = {}
                for name in ("Wq", "We", "Wg", "Wb"):
                    f = setup_pool.tile([BK, BK], dt.float32, tag="ldf32_" + name)
                    nc.sync.dma_start(f[:], w_dram[name][:])
                    d[name] = f
                bq_f = statics.tile([128, 1], dt.float32)
                nc.sync.dma_start(bq_f[:], w_dram["bq"][:].rearrange("(p u) -> p u", u=1))
                d["bq_f"] = bq_f
                for name in ("bk", "bv", "bg", "bb"):
                    f = setup_pool.tile([1, 128], dt.float32, tag="ldrow_" + name)
                    nc.sync.dma_start(f[:], w_dram[name][:].rearrange("(u d) -> u d", u=1))
                    d["row_" + name] = f
                hl_f = setup_pool.tile([T_TEXT, D_LLM], dt.float32, tag="ldf32_hl")
                nc.sync.dma_start(hl_f[:], hl_dram[:])
                d["hl_f"] = hl_f
                wk_f = setup_pool.tile([BK, D_LLM], dt.float32, tag="ldf32_Wk")
                nc.sync.dma_start(wk_f[:], w_dram["Wk"][:])
                d["wk_f"] = wk_f
                return d

            def setup_dma_b():
                wv_f = setup_pool.tile([BK, D_LLM], dt.float32, tag="ldf32_Wv")
                nc.sync.dma_start(wv_f[:], w_dram["Wv"][:])
                return wv_f

            # ---- setup compute part A: square weights, bias rows, k chain
            def setup_compute_a(d):
                sqT16 = {}
                for name, scl in [("Wq", None), ("We", None), ("Wg", 0.5), ("Wb", 0.5)]:
                    w16 = statics.tile([BK, BK], dt.bfloat16, tag="c16_" + name)
                    if scl is None:
                        nc.scalar.activation(w16[:], d[name][:], AF.Copy)
                    else:
                        nc.scalar.activation(w16[:], d[name][:], AF.Identity, scale=scl)
                    ps = attn_ps.tile([128, 128], dt.bfloat16, tag="aps")
                    nc.tensor.transpose(ps[:], w16[:], ident16[:])
                    t16 = statics.tile([BK, BK], dt.bfloat16, tag="T16_" + name)
                    nc.scalar.activation(t16[:], ps[:], AF.Copy)
                    sqT16[name] = t16

                brow16 = {}
                for name, scl in (("bk", None), ("bv", None), ("bg", 0.5), ("bb", 0.5)):
                    r16 = statics.tile([1, 128], dt.bfloat16, tag="r16_" + name)
                    if scl is None:
                        nc.scalar.activation(r16[:], d["row_" + name][:], AF.Copy)
                    else:
                        nc.scalar.activation(r16[:], d["row_" + name][:], AF.Identity,
                                             scale=scl)
                    brow16[name] = r16

                # k chain: fp32 transposes of hl/Wk chunks, bf16 copies, matmuls
                # NOTE: Pool/GPSIMD cannot read PSUM on real HW, so PSUM->SBUF
                # copies go on ACT/DVE only.
                hlT_c = []
                for c in range(NCH):
                    ps = attn_ps.tile([128, T_TEXT], dt.float32, tag="aps")
                    nc.tensor.transpose(ps[:], d["hl_f"][:, bass.ts(c, 128)],
                                        ident32[:T_TEXT, :T_TEXT])
                    sb = setup_pool.tile([128, T_TEXT], dt.bfloat16, tag=f"hlT{c}")
                    if c % 2 == 0:
                        nc.scalar.activation(sb[:], ps[:], AF.Copy)
                    else:
                        nc.vector.tensor_copy(sb[:], ps[:])
                    hlT_c.append(sb)
                wkT_c = []
                for c in range(NCH):
                    ps2 = attn_ps.tile([128, 128], dt.float32, tag="aps")
                    nc.tensor.transpose(ps2[:], d["wk_f"][:, bass.ts(c, 128)], ident32[:])
                    sb2 = setup_pool.tile([128, 128], dt.bfloat16, tag=f"wkT{c}")
                    if c % 2 == 0:
                        nc.vector.tensor_copy(sb2[:], ps2[:])
                    else:
                        nc.scalar.activation(sb2[:], ps2[:], AF.Copy)
                    wkT_c.append(sb2)
                k_ps = equi_ps.tile([T_TEXT, BK], dt.float32, tag="eq")
                for c in range(NCH):
                    nc.tensor.matmul(k_ps[:], hlT_c[c][:], wkT_c[c][:],
                                     start=(c == 0), stop=False)
                nc.tensor.matmul(k_ps[:], ones16[:, :T_TEXT], brow16["bk"][:],
                                 start=False, stop=True)
                k16 = statics.tile([T_TEXT, BK], dt.bfloat16)
                nc.scalar.activation(k16[:], k_ps[:], AF.Copy)
                kT_ps = attn_ps.tile([BK, T_TEXT], dt.bfloat16, tag="aps")
                nc.tensor.transpose(kT_ps[:], k16[:], ident16[:T_TEXT, :T_TEXT])
                kT16 = statics.tile([BK, T_TEXT], dt.bfloat16)
                nc.scalar.activation(kT16[:], kT_ps[:], AF.Copy)
                setup_out.update(
                    WeT16=sqT16["We"], WqT16=sqT16["Wq"], WgT16=sqT16["Wg"],
                    WbT16=sqT16["Wb"], bq_f=d["bq_f"], brow16=brow16,
                    k16=k16, kT16=kT16, hlT_c=hlT_c)

            # ---- setup compute part B: v chain
            def setup_compute_b(wv_f):
                hlT_c = setup_out["hlT_c"]
                brow16 = setup_out["brow16"]
                wvT_c = []
                for c in range(NCH):
                    ps2 = attn_ps.tile([128, 128], dt.float32, tag="aps")
                    nc.tensor.transpose(ps2[:], wv_f[:, bass.ts(c, 128)], ident32[:])
                    sb2 = setup_pool.tile([128, 128], dt.bfloat16, tag=f"wvT{c}")
                    if c % 2 == 0:
                        nc.vector.tensor_copy(sb2[:], ps2[:])
                    else:
                        nc.scalar.activation(sb2[:], ps2[:], AF.Copy)
                    wvT_c.append(sb2)
                v_ps = equi_ps.tile([T_TEXT, BK], dt.float32, tag="eq")
                for c in range(NCH):
                    nc.tensor.matmul(v_ps[:], hlT_c[c][:], wvT_c[c][:],
                                     start=(c == 0), stop=False)
                nc.tensor.matmul(v_ps[:], ones16[:, :T_TEXT], brow16["bv"][:],
                                 start=False, stop=True)
                v16 = statics.tile([T_TEXT, BK], dt.bfloat16)
                nc.scalar.activation(v16[:], v_ps[:], AF.Copy)
                setup_out["v16"] = v16

            # ---------------- main loop over token chunks ----------------
            h3 = h_dram[:].rearrange("n (c s p) d -> n c p s d", p=128, s=NSUB)
            o3 = out_dram[:].rearrange("n (c s p) d -> n c p s d", p=128, s=NSUB)

            # hT copy engine split: g -> engine (no Pool: GPSIMD can't read
            # PSUM; only add/sub/mult tensor_tensor runs on GPSIMD ucode)
            HT_ENG = ["act", "act", "act", "act", "act", "dve", "dve", "dve"]

            def copy_op(eng, dst, src):
                if eng == "act":
                    nc.scalar.activation(dst, src, AF.Copy)
                elif eng == "dve":
                    nc.vector.tensor_copy(dst, src)
                else:
                    nc.gpsimd.tensor_copy(dst, src)

            def phase_a_dma(n, cch, s0, ns):
                h_f = hin_pool.tile([128, NSUB, D], dt.float32, tag="h_f")
                nc.sync.dma_start(h_f[:, 0:ns], h3[n, cch, :, s0:s0 + ns])
                return h_f

            def phase_a_compute(h_f, ns):
                tck = 128 * ns
                hT16 = ht_pool.tile([128, G, TC], dt.bfloat16, tag="hT16")
                for g in range(G):
                    htp = htps_ps.tile([128, TC], dt.float32, tag="htp")
                    for s in range(ns):
                        nc.tensor.transpose(
                            htp[:, bass.ts(s, 128)],
                            h_f[:, s, bass.ts(g, 128)],
                            ident32[:],
                        )
                    copy_op(HT_ENG[g], hT16[:, g, 0:tck], htp[:, 0:tck])

                # group-max tree on DVE (GPSIMD tensor_tensor lacks max)
                m4 = tree_pool.tile([128, 4, TC], dt.bfloat16, tag="m4")
                nc.vector.tensor_max(m4[:, :, 0:tck], hT16[:, 0:4, 0:tck],
                                     hT16[:, 4:8, 0:tck])
                m2 = tree_pool.tile([128, 2, TC], dt.bfloat16, tag="m2")
                nc.vector.tensor_max(m2[:, :, 0:tck], m4[:, 0:2, 0:tck],
                                     m4[:, 2:4, 0:tck])
                invT = inv_pool.tile([128, TC], dt.bfloat16, tag="invT")
                nc.vector.tensor_max(invT[:, 0:tck], m2[:, 0, 0:tck],
                                     m2[:, 1, 0:tck])
                return h_f, hT16, invT

            def phase_b(n, cch, s0, ns, h_f, hT16, invT):
                tck = 128 * ns
                # ---- attention chain (latency-critical; PE first) ----
                qt_psum = attn_ps.tile([128, TC], dt.float32, tag="aps")
                nc.tensor.matmul(qt_psum[:, 0:tck], setup_out["WqT16"][:],
                                 invT[:, 0:tck], start=True, stop=True)
                qT16 = ssb_pool.tile([128, TC], dt.bfloat16, tag="qT")
                nc.scalar.activation(qT16[:, 0:tck], qt_psum[:, 0:tck], AF.Identity,
                                     bias=setup_out["bq_f"][:])

                lg_ps = attn_ps.tile([128, NSUB, T_TEXT], dt.float32, tag="aps")
                for s in range(ns):
                    nc.tensor.matmul(lg_ps[:, s], qT16[:, bass.ts(s, 128)],
                                     setup_out["kT16"][:], start=True, stop=True)
                ae16 = ssb_pool.tile([128, NSUB, T_TEXT], dt.bfloat16, tag="ae")
                nc.scalar.activation(ae16[:, 0:ns], lg_ps[:, 0:ns], AF.Exp,
                                     scale=float(SCALE))
                denoms = ssb_pool.tile([128, NSUB], dt.float32, tag="den")
                nc.vector.tensor_reduce(denoms[:, 0:ns], ae16[:, 0:ns],
                                        axis=mybir.AxisListType.X,
                                        op=mybir.AluOpType.add)
                recips = ssb_pool.tile([128, NSUB], dt.float32, tag="rcp")
                nc.vector.reciprocal(recips[:, 0:ns], denoms[:, 0:ns])
                aeN = ssb_pool.tile([128, NSUB, T_TEXT], dt.bfloat16, tag="aeN")
                nc.vector.tensor_mul(
                    aeN[:, 0:ns], ae16[:, 0:ns],
                    recips[:, 0:ns, None].broadcast_to([128, ns, T_TEXT]))

                at_ps = attn_ps.tile([T_TEXT, NSUB, 128], dt.bfloat16, tag="aps")
                for s in range(ns):
                    nc.tensor.transpose(at_ps[:, s], aeN[:, s], ident16[:])
                at16 = ssb_pool.tile([T_TEXT, NSUB, 128], dt.bfloat16, tag="at16")
                nc.vector.tensor_copy(at16[:, 0:ns], at_ps[:, 0:ns])

                # ctx [t, s, d] (for gate) and ctxT [d, s, t] (for gm/bt)
                ctx_ps = attn_ps.tile([128, NSUB, BK], dt.float32, tag="aps")
                ctxT_ps = attn_ps.tile([128, NSUB, BK], dt.float32, tag="aps")
                for s in range(ns):
                    nc.tensor.matmul(ctx_ps[:, s], at16[:, s], setup_out["v16"][:],
                                     start=True, stop=True)
                    nc.tensor.matmul(ctxT_ps[:, s], setup_out["v16"][:], at16[:, s],
                                     start=True, stop=True)
                gate16 = ssb_pool.tile([128, NSUB, BK], dt.bfloat16, tag="gate")
                nc.scalar.activation(gate16[:, 0:ns], ctx_ps[:, 0:ns], AF.Tanh,
                                     scale=0.5)
                ctxT16 = ssb_pool.tile([128, NSUB, BK], dt.bfloat16, tag="ctxT16")
                nc.vector.tensor_copy(ctxT16[:, 0:ns], ctxT_ps[:, 0:ns])

                gm_ps = attn_ps.tile([128, NSUB, BK], dt.float32, tag="aps")
                bt_ps = attn_ps.tile([128, NSUB, BK], dt.float32, tag="aps")
                for s in range(ns):
                    nc.tensor.matmul(gm_ps[:, s], ctxT16[:, s], setup_out["WgT16"][:],
                                     start=True, stop=False)
                    nc.tensor.matmul(gm_ps[:, s], ones16[:], setup_out["brow16"]["bg"][:],
                                     start=False, stop=True)
                    nc.tensor.matmul(bt_ps[:, s], ctxT16[:, s], setup_out["WbT16"][:],
                                     start=True, stop=False)
                    nc.tensor.matmul(bt_ps[:, s], ones16[:], setup_out["brow16"]["bb"][:],
                                     start=False, stop=True)
                # P = (gate'+1)*(gamma/2), Q = (gate'+1)*(beta/2) from PSUM
                P16 = ssb_pool.tile([128, NSUB, 128], dt.bfloat16, tag="P")
                Q16 = ssb_pool.tile([128, NSUB, 128], dt.bfloat16, tag="Q")
                nc.vector.scalar_tensor_tensor(
                    P16[:, 0:ns], gate16[:, 0:ns], 1.0, gm_ps[:, 0:ns],
                    op0=mybir.AluOpType.add, op1=mybir.AluOpType.mult)
                nc.vector.scalar_tensor_tensor(
                    Q16[:, 0:ns], gate16[:, 0:ns], 1.0, bt_ps[:, 0:ns],
                    op0=mybir.AluOpType.add, op1=mybir.AluOpType.mult)

                # ---- equi matmuls + modulation, per sub-tile pipelined:
                # corr(s) = eq(s)*P (DVE; via ACT bf16 copy except last s,
                # which multiplies straight from PSUM), += Q on Pool,
                # h_f(s) += corr(s) on Pool, DMA out s.
                eqb16 = eqb_pool.tile([128, NSUB, G, BK], dt.bfloat16, tag="eqb")
                corr = eqb16
                hc = h_f[:].rearrange("p s (g o) -> p s g o", g=G)
                for s in range(ns):
                    eq_ps = equi_ps.tile([128, G, BK], dt.float32, tag="eq")
                    for g in range(G):
                        nc.tensor.matmul(
                            eq_ps[:, g],
                            hT16[:, g, bass.ts(s, 128)],
                            setup_out["WeT16"][:],
                            start=True, stop=True,
                        )
                    nc.scalar.activation(eqb16[:, s], eq_ps[:], AF.Copy)
                    nc.vector.tensor_mul(
                        corr[:, s], eqb16[:, s],
                        P16[:, s, None, :].broadcast_to([128, G, BK]),
                    )
                    nc.gpsimd.tensor_add(
                        corr[:, s], corr[:, s],
                        Q16[:, s, None, :].broadcast_to([128, G, BK]),
                    )
                    # fp32 residual in place into h_f
                    nc.gpsimd.tensor_add(hc[:, s], corr[:, s], hc[:, s])
                    nc.sync.dma_start(o3[n, cch, :, s0 + s], h_f[:, s])

            # ---------------- program order (ramp-optimized) ----------------
            NCH_TOT = N_EQUI * (T // TC)
            # full chunks, with the last split in two for a shorter tail
            chunks = [(*divmod(k, T // TC), 0, NSUB) for k in range(NCH_TOT - 1)]
            lastn, lastc = divmod(NCH_TOT - 1, T // TC)
            half = NSUB // 2
            chunks.append((lastn, lastc, 0, half))
            chunks.append((lastn, lastc, half, half))
            NK = len(chunks)

            abuf = {}
            hf0 = phase_a_dma(*chunks[0])
            d_a = setup_dma_a()
            setup_compute_a(d_a)
            abuf[0] = phase_a_compute(hf0, chunks[0][3])
            wv_f = setup_dma_b()
            setup_compute_b(wv_f)
            hf1 = phase_a_dma(*chunks[1])
            abuf[1] = phase_a_compute(hf1, chunks[1][3])
            for k in range(NK):
                phase_b(*chunks[k], *abuf.pop(k))
                if k + 2 < NK:
                    hfk = phase_a_dma(*chunks[k + 2])
                    abuf[k + 2] = phase_a_compute(hfk, chunks[k + 2][3])

    nc.compile()
    return nc


_NC_CACHE = None


def _get_nc():
    global _NC_CACHE
    if _NC_CACHE is None:
        _NC_CACHE = _build_program()
    return _NC_CACHE


def kernel(**inputs) -> np.ndarray:
    h_prime = np.ascontiguousarray(inputs["h_prime"], dtype=np.float32)
    h_llm = np.ascontiguousarray(inputs["h_llm"], dtype=np.float32)
    wnames = ["Wq", "Wk", "Wv", "Wg", "Wb", "We", "bq", "bk", "bv", "bg", "bb"]
    wmap = {n: np.ascontiguousarray(inputs[n], dtype=np.float32) for n in wnames}

    nc = _get_nc()
    in_maps = []
    for b in range(B):
        m = {"h": h_prime[b], "hl": h_llm[b]}
        m.update(wmap)
        in_maps.append(m)

    res = run_bass_kernel_spmd(nc, in_maps, list(range(B)), trace=TRACE,
                               **(RUN_KWARGS or {}))
    global LAST_EXEC_TIME_NS, LAST_RESULT
    LAST_EXEC_TIME_NS = res.exec_time_ns
    LAST_RESULT = res
    out = np.stack([res.results[b]["out"] for b in range(B)], axis=0)
    return out.astype(np.float32)


LAST_EXEC_TIME_NS = None
LAST_RESULT = None
TRACE = False
RUN_KWARGS = None


if __name__ == "__main__":
    nc = _get_nc()
    print("program built ok")


# revision 81
# speedup vs baseline: 1.0071x; 1.0071x over previous
"""Trainium2 Bass kernel for nn_EquiAdapter (8-core data-parallel over batch).

Math (per batch b, equi-channel n, token t):
  h_slices = h_prime.reshape(B, n_equi, T, 8, 128)
  equi_out[t,g,o] = sum_i h[t,g,i] * We[o,i]
  inv_feat = max_g h_slices                       [t, 128]
  q = inv_feat @ Wq.T + bq ; k = h_llm @ Wk.T + bk ; v = h_llm @ Wv.T + bv
  attn = softmax(q @ k.T / sqrt(128))             [t, 64]
  ctx = attn @ v ; gate = sigmoid(ctx)
  gamma = ctx @ Wg.T + bg ; beta = ctx @ Wb.T + bb
  out = h_prime + bcast_g(gate) * (bcast_g(gamma)*equi_out + bcast_g(beta))

v3: engine-balanced + ramp-optimized, targeting the ~115us DMA floor.
  - DMA issue order interleaves setup weights with the first h chunks so the
    k-projection chain (needed by the first attention) completes early.
  - Attention smalls: exp once over all 4 sub-tiles; attn weights normalized
    (one DVE op) instead of per-s ctx scaling; ctxT computed directly by PE
    (v16 stationary); P/Q built straight from gamma/beta PSUM.
  - equi -> fp32 PSUM (one 2-bank tile) -> ACT copies to bf16 -> one big DVE
    2x multiply (*P) and one Pool 2x add (+Q) in place; residual split
    DVE/Pool.  0.5 escape-scaling folded into Wg/Wb/bg/bb at setup.
"""

import sys

sys.path.insert(0, "/opt/trn_rl_repo")

import numpy as np

import concourse.bass as bass
import concourse.bacc as bacc
import concourse.tile as tile
from concourse import mybir
from concourse import masks
from concourse.bass_utils import run_bass_kernel_spmd

dt = mybir.dt
AF = mybir.ActivationFunctionType

B, N_EQUI, T, D = 8, 4, 1024, 1024
T_TEXT, D_LLM = 64, 2048
G, BK = 8, 128  # n_group, blocks
SCALE = BK ** (-0.5)
TC = 512  # token chunk
NSUB = TC // 128  # 4
NCHUNK = (N_EQUI * T) // TC  # 8 chunks per core


def _build_program():
    nc = bacc.Bacc("TRN2", target_bir_lowering=False, debug=False)

    h_dram = nc.declare_dram_parameter("h", [N_EQUI, T, D], dt.float32, isOutput=False)
    hl_dram = nc.declare_dram_parameter("hl", [T_TEXT, D_LLM], dt.float32, isOutput=False)
    w_dram = {}
    for name, shape in [
        ("Wq", [BK, BK]), ("Wk", [BK, D_LLM]), ("Wv", [BK, D_LLM]),
        ("Wg", [BK, BK]), ("Wb", [BK, BK]), ("We", [BK, BK]),
        ("bq", [BK]), ("bk", [BK]), ("bv", [BK]), ("bg", [BK]), ("bb", [BK]),
    ]:
        w_dram[name] = nc.declare_dram_parameter(name, shape, dt.float32, isOutput=False)
    out_dram = nc.declare_dram_parameter("out", [N_EQUI, T, D], dt.float32, isOutput=True)

    import contextlib
    with tile.TileContext(nc) as tc:
        with contextlib.ExitStack() as es:
            statics = es.enter_context(tc.tile_pool(name="static", bufs=1))
            setup_pool = es.enter_context(tc.tile_pool(name="setup_f32", bufs=1))
            # PSUM: htps 2 banks + eq 2 banks + attn 4 banks = 8
            htps_ps = es.enter_context(tc.tile_pool(name="htps_ps", bufs=2, space="PSUM"))
            equi_ps = es.enter_context(tc.tile_pool(name="equi_ps", bufs=1, space="PSUM"))
            attn_ps = es.enter_context(tc.tile_pool(name="attn_ps", bufs=4, space="PSUM"))

            hin_pool = es.enter_context(tc.tile_pool(name="hin", bufs=5))
            ht_pool = es.enter_context(tc.tile_pool(name="ht16", bufs=3))
            tree_pool = es.enter_context(tc.tile_pool(name="tree", bufs=1))
            inv_pool = es.enter_context(tc.tile_pool(name="invf", bufs=2))
            eqb_pool = es.enter_context(tc.tile_pool(name="eqb", bufs=3))
            ssb_pool = es.enter_context(tc.tile_pool(name="small_sb", bufs=2))

            ident16 = statics.tile([128, 128], dt.bfloat16)
            masks.make_identity(nc, ident16[:])
            ident32 = statics.tile([128, 128], dt.float32)
            masks.make_identity(nc, ident32[:])
            ones16 = statics.tile([1, 128], dt.bfloat16)
            nc.gpsimd.memset(ones16[:], 1.0)

            setup_out = {}
            NCH = D_LLM // 128  # 16 contraction chunks

            # ---- setup DMA part A: everything the attention k-chain and the
            # equi/q paths need (small weights + h_llm + Wk)
            def setup_dma_a():
                d = {}
                for name in ("Wq", "We", "Wg", "Wb"):
                    f = setup_pool.tile([BK, BK], dt.float32, tag="ldf32_" + name)
                    nc.sync.dma_start(f[:], w_dram[name][:])
                    d[name] = f
                bq_f = statics.tile([128, 1], dt.float32)
                nc.sync.dma_start(bq_f[:], w_dram["bq"][:].rearrange("(p u) -> p u", u=1))
                d["bq_f"] = bq_f
                for name in ("bk", "bv", "bg", "bb"):
                    f = setup_pool.tile([1, 128], dt.float32, tag="ldrow_" + name)
                    nc.sync.dma_start(f[:], w_dram[name][:].rearrange("(u d) -> u d", u=1))
                    d["row_" + name] = f
                hl_f = setup_pool.tile([T_TEXT, D_LLM], dt.float32, tag="ldf32_hl")
                nc.sync.dma_start(hl_f[:], hl_dram[:])
                d["hl_f"] = hl_f
                wk_f = setup_pool.tile([BK, D_LLM], dt.float32, tag="ldf32_Wk")
                nc.sync.dma_start(wk_f[:], w_dram["Wk"][:])
                d["wk_f"] = wk_f
                return d

            def setup_dma_b():
                wv_f = setup_pool.tile([BK, D_LLM], dt.float32, tag="ldf32_Wv")
                nc.sync.dma_start(wv_f[:], w_dram["Wv"][:])
                return wv_f

            # ---- setup compute part A: square weights, bias rows, k chain
            def setup_compute_a(d):
                sqT16 = {}
                for name, scl in [("Wq", None), ("We", None), ("Wg", 0.5), ("Wb", 0.5)]:
                    w16 = statics.tile([BK, BK], dt.bfloat16, tag="c16_" + name)
                    if scl is None:
                        nc.scalar.activation(w16[:], d[name][:], AF.Copy)
                    else:
                        nc.scalar.activation(w16[:], d[name][:], AF.Identity, scale=scl)
                    ps = attn_ps.tile([128, 128], dt.bfloat16, tag="aps")
                    nc.tensor.transpose(ps[:], w16[:], ident16[:])
                    t16 = statics.tile([BK, BK], dt.bfloat16, tag="T16_" + name)
                    nc.scalar.activation(t16[:], ps[:], AF.Copy)
                    sqT16[name] = t16

                brow16 = {}
                for name, scl in (("bk", None), ("bv", None), ("bg", 0.5), ("bb", 0.5)):
                    r16 = statics.tile([1, 128], dt.bfloat16, tag="r16_" + name)
                    if scl is None:
                        nc.scalar.activation(r16[:], d["row_" + name][:], AF.Copy)
                    else:
                        nc.scalar.activation(r16[:], d["row_" + name][:], AF.Identity,
                                             scale=scl)
                    brow16[name] = r16

                # k chain: fp32 transposes of hl/Wk chunks, bf16 copies, matmuls
                # NOTE: Pool/GPSIMD cannot read PSUM on real HW, so PSUM->SBUF
                # copies go on ACT/DVE only.
                hlT_c = []
                for c in range(NCH):
                    ps = attn_ps.tile([128, T_TEXT], dt.float32, tag="aps")
                    nc.tensor.transpose(ps[:], d["hl_f"][:, bass.ts(c, 128)],
                                        ident32[:T_TEXT, :T_TEXT])
                    sb = setup_pool.tile([128, T_TEXT], dt.bfloat16, tag=f"hlT{c}")
                    if c % 2 == 0:
                        nc.scalar.activation(sb[:], ps[:], AF.Copy)
                    else:
                        nc.vector.tensor_copy(sb[:], ps[:])
                    hlT_c.append(sb)
                wkT_c = []
                for c in range(NCH):
                    ps2 = attn_ps.tile([128, 128], dt.float32, tag="aps")
                    nc.tensor.transpose(ps2[:], d["wk_f"][:, bass.ts(c, 128)], ident32[:])
                    sb2 = setup_pool.tile([128, 128], dt.bfloat16, tag=f"wkT{c}")
                    if c % 2 == 0:
                        nc.vector.tensor_copy(sb2[:], ps2[:])
                    else:
                        nc.scalar.activation(sb2[:], ps2[:], AF.Copy)
                    wkT_c.append(sb2)
                k_ps = equi_ps.tile([T_TEXT, BK], dt.float32, tag="eq")
                for c in range(NCH):
                    nc.tensor.matmul(k_ps[:], hlT_c[c][:], wkT_c[c][:],
                                     start=(c == 0), stop=False)
                nc.tensor.matmul(k_ps[:], ones16[:, :T_TEXT], brow16["bk"][:],
                                 start=False, stop=True)
                k16 = statics.tile([T_TEXT, BK], dt.bfloat16)
                nc.scalar.activation(k16[:], k_ps[:], AF.Copy)
                kT_ps = attn_ps.tile([BK, T_TEXT], dt.bfloat16, tag="aps")
                nc.tensor.transpose(kT_ps[:], k16[:], ident16[:T_TEXT, :T_TEXT])
                kT16 = statics.tile([BK, T_TEXT], dt.bfloat16)
                nc.scalar.activation(kT16[:], kT_ps[:], AF.Copy)
                setup_out.update(
                    WeT16=sqT16["We"], WqT16=sqT16["Wq"], WgT16=sqT16["Wg"],
                    WbT16=sqT16["Wb"], bq_f=d["bq_f"], brow16=brow16,
                    k16=k16, kT16=kT16, hlT_c=hlT_c)

            # ---- setup compute part B: v chain
            def setup_compute_b(wv_f):
                hlT_c = setup_out["hlT_c"]
                brow16 = setup_out["brow16"]
                wvT_c = []
                for c in range(NCH):
                    ps2 = attn_ps.tile([128, 128], dt.float32, tag="aps")
                    nc.tensor.transpose(ps2[:], wv_f[:, bass.ts(c, 128)], ident32[:])
                    sb2 = setup_pool.tile([128, 128], dt.bfloat16, tag=f"wvT{c}")
                    if c % 2 == 0:
                        nc.vector.tensor_copy(sb2[:], ps2[:])
                    else:
                        nc.scalar.activation(sb2[:], ps2[:], AF.Copy)
                    wvT_c.append(sb2)
                v_ps = equi_ps.tile([T_TEXT, BK], dt.float32, tag="eq")
                for c in range(NCH):
                    nc.tensor.matmul(v_ps[:], hlT_c[c][:], wvT_c[c][:],
                                     start=(c == 0), stop=False)
                nc.tensor.matmul(v_ps[:], ones16[:, :T_TEXT], brow16["bv"][:],
                                 start=False, stop=True)
                v16 = statics.tile([T_TEXT, BK], dt.bfloat16)
                nc.scalar.activation(v16[:], v_ps[:], AF.Copy)
                setup_out["v16"] = v16

            # ---------------- main loop over token chunks ----------------
            h3 = h_dram[:].rearrange("n (c s p) d -> n c p s d", p=128, s=NSUB)
            o3 = out_dram[:].rearrange("n (c s p) d -> n c p s d", p=128, s=NSUB)

            # hT copy engine split: g -> engine (no Pool: GPSIMD can't read
            # PSUM; only add/sub/mult tensor_tensor runs on GPSIMD ucode)
            HT_ENG = ["act", "act", "act", "act", "act", "dve", "dve", "dve"]

            def copy_op(eng, dst, src):
                if eng == "act":
                    nc.scalar.activation(dst, src, AF.Copy)
                elif eng == "dve":
                    nc.vector.tensor_copy(dst, src)
                else:
                    nc.gpsimd.tensor_copy(dst, src)

            def phase_a_dma(n, cch, s0, ns):
                h_f = hin_pool.tile([128, NSUB, D], dt.float32, tag="h_f")
                nc.sync.dma_start(h_f[:, 0:ns], h3[n, cch, :, s0:s0 + ns])
                return h_f

            def phase_a_compute(h_f, ns):
                tck = 128 * ns
                hT16 = ht_pool.tile([128, G, TC], dt.bfloat16, tag="hT16")
                for g in range(G):
                    htp = htps_ps.tile([128, TC], dt.float32, tag="htp")
                    for s in range(ns):
                        nc.tensor.transpose(
                            htp[:, bass.ts(s, 128)],
                            h_f[:, s, bass.ts(g, 128)],
                            ident32[:],
                        )
                    copy_op(HT_ENG[g], hT16[:, g, 0:tck], htp[:, 0:tck])

                # group-max tree on DVE (GPSIMD tensor_tensor lacks max)
                m4 = tree_pool.tile([128, 4, TC], dt.bfloat16, tag="m4")
                nc.vector.tensor_max(m4[:, :, 0:tck], hT16[:, 0:4, 0:tck],
                                     hT16[:, 4:8, 0:tck])
                m2 = tree_pool.tile([128, 2, TC], dt.bfloat16, tag="m2")
                nc.vector.tensor_max(m2[:, :, 0:tck], m4[:, 0:2, 0:tck],
                                     m4[:, 2:4, 0:tck])
                invT = inv_pool.tile([128, TC], dt.bfloat16, tag="invT")
                nc.vector.tensor_max(invT[:, 0:tck], m2[:, 0, 0:tck],
                                     m2[:, 1, 0:tck])
                return h_f, hT16, invT

            def phase_b(n, cch, s0, ns, h_f, hT16, invT, final=False):
                tck = 128 * ns
                # ---- attention chain (latency-critical; PE first) ----
                qt_psum = attn_ps.tile([128, TC], dt.float32, tag="aps")
                nc.tensor.matmul(qt_psum[:, 0:tck], setup_out["WqT16"][:],
                                 invT[:, 0:tck], start=True, stop=True)
                qT16 = ssb_pool.tile([128, TC], dt.bfloat16, tag="qT")
                nc.scalar.activation(qT16[:, 0:tck], qt_psum[:, 0:tck], AF.Identity,
                                     bias=setup_out["bq_f"][:])

                lg_ps = attn_ps.tile([128, NSUB, T_TEXT], dt.float32, tag="aps")
                for s in range(ns):
                    nc.tensor.matmul(lg_ps[:, s], qT16[:, bass.ts(s, 128)],
                                     setup_out["kT16"][:], start=True, stop=True)
                ae16 = ssb_pool.tile([128, NSUB, T_TEXT], dt.bfloat16, tag="ae")
                nc.scalar.activation(ae16[:, 0:ns], lg_ps[:, 0:ns], AF.Exp,
                                     scale=float(SCALE))
                denoms = ssb_pool.tile([128, NSUB], dt.float32, tag="den")
                nc.vector.tensor_reduce(denoms[:, 0:ns], ae16[:, 0:ns],
                                        axis=mybir.AxisListType.X,
                                        op=mybir.AluOpType.add)
                recips = ssb_pool.tile([128, NSUB], dt.float32, tag="rcp")
                nc.vector.reciprocal(recips[:, 0:ns], denoms[:, 0:ns])
                aeN = ssb_pool.tile([128, NSUB, T_TEXT], dt.bfloat16, tag="aeN")
                nc.vector.tensor_mul(
                    aeN[:, 0:ns], ae16[:, 0:ns],
                    recips[:, 0:ns, None].broadcast_to([128, ns, T_TEXT]))

                at_ps = attn_ps.tile([T_TEXT, NSUB, 128], dt.bfloat16, tag="aps")
                for s in range(ns):
                    nc.tensor.transpose(at_ps[:, s], aeN[:, s], ident16[:])
                at16 = ssb_pool.tile([T_TEXT, NSUB, 128], dt.bfloat16, tag="at16")
                nc.vector.tensor_copy(at16[:, 0:ns], at_ps[:, 0:ns])

                # ctx [t, s, d] (for gate) and ctxT [d, s, t] (for gm/bt)
                ctx_ps = attn_ps.tile([128, NSUB, BK], dt.float32, tag="aps")
                ctxT_ps = attn_ps.tile([128, NSUB, BK], dt.float32, tag="aps")
                for s in range(ns):
                    nc.tensor.matmul(ctx_ps[:, s], at16[:, s], setup_out["v16"][:],
                                     start=True, stop=True)
                    nc.tensor.matmul(ctxT_ps[:, s], setup_out["v16"][:], at16[:, s],
                                     start=True, stop=True)
                gate16 = ssb_pool.tile([128, NSUB, BK], dt.bfloat16, tag="gate")
                nc.scalar.activation(gate16[:, 0:ns], ctx_ps[:, 0:ns], AF.Tanh,
                                     scale=0.5)
                ctxT16 = ssb_pool.tile([128, NSUB, BK], dt.bfloat16, tag="ctxT16")
                nc.vector.tensor_copy(ctxT16[:, 0:ns], ctxT_ps[:, 0:ns])

                gm_ps = attn_ps.tile([128, NSUB, BK], dt.float32, tag="aps")
                bt_ps = attn_ps.tile([128, NSUB, BK], dt.float32, tag="aps")
                for s in range(ns):
                    nc.tensor.matmul(gm_ps[:, s], ctxT16[:, s], setup_out["WgT16"][:],
                                     start=True, stop=False)
                    nc.tensor.matmul(gm_ps[:, s], ones16[:], setup_out["brow16"]["bg"][:],
                                     start=False, stop=True)
                    nc.tensor.matmul(bt_ps[:, s], ctxT16[:, s], setup_out["WbT16"][:],
                                     start=True, stop=False)
                    nc.tensor.matmul(bt_ps[:, s], ones16[:], setup_out["brow16"]["bb"][:],
                                     start=False, stop=True)
                # P = (gate'+1)*(gamma/2), Q = (gate'+1)*(beta/2) from PSUM
                P16 = ssb_pool.tile([128, NSUB, 128], dt.bfloat16, tag="P")
                Q16 = ssb_pool.tile([128, NSUB, 128], dt.bfloat16, tag="Q")
                nc.vector.scalar_tensor_tensor(
                    P16[:, 0:ns], gate16[:, 0:ns], 1.0, gm_ps[:, 0:ns],
                    op0=mybir.AluOpType.add, op1=mybir.AluOpType.mult)
                nc.vector.scalar_tensor_tensor(
                    Q16[:, 0:ns], gate16[:, 0:ns], 1.0, bt_ps[:, 0:ns],
                    op0=mybir.AluOpType.add, op1=mybir.AluOpType.mult)

                # ---- equi matmuls + modulation, per sub-tile pipelined:
                # corr(s) = eq(s)*P (DVE; via ACT bf16 copy except last s,
                # which multiplies straight from PSUM), += Q on Pool,
                # h_f(s) += corr(s) on Pool, DMA out s.
                eqb16 = eqb_pool.tile([128, NSUB, G, BK], dt.bfloat16, tag="eqb")
                corr = eqb16
                hc = h_f[:].rearrange("p s (g o) -> p s g o", g=G)
                for s in range(ns):
                    eq_ps = equi_ps.tile([128, G, BK], dt.float32, tag="eq")
                    for g in range(G):
                        nc.tensor.matmul(
                            eq_ps[:, g],
                            hT16[:, g, bass.ts(s, 128)],
                            setup_out["WeT16"][:],
                            start=True, stop=True,
                        )
                    nc.scalar.activation(eqb16[:, s], eq_ps[:], AF.Copy)
                    nc.vector.tensor_mul(
                        corr[:, s], eqb16[:, s],
                        P16[:, s, None, :].broadcast_to([128, G, BK]),
                    )
                    # final half-chunk: give each sub-tile its own engine
                    # chain (Pool / DVE) so the two pieces drain in parallel
                    mod_eng = nc.vector if (final and s == 1) else nc.gpsimd
                    mod_eng.tensor_add(
                        corr[:, s], corr[:, s],
                        Q16[:, s, None, :].broadcast_to([128, G, BK]),
                    )
                    # fp32 residual in place into h_f
                    mod_eng.tensor_add(hc[:, s], corr[:, s], hc[:, s])
                    nc.sync.dma_start(o3[n, cch, :, s0 + s], h_f[:, s])

            # ---------------- program order (ramp-optimized) ----------------
            NCH_TOT = N_EQUI * (T // TC)
            # full chunks, with the last split in two for a shorter tail
            chunks = [(*divmod(k, T // TC), 0, NSUB) for k in range(NCH_TOT - 1)]
            lastn, lastc = divmod(NCH_TOT - 1, T // TC)
            half = NSUB // 2
            chunks.append((lastn, lastc, 0, half))
            chunks.append((lastn, lastc, half, half))
            NK = len(chunks)

            abuf = {}
            hf0 = phase_a_dma(*chunks[0])
            d_a = setup_dma_a()
            setup_compute_a(d_a)
            abuf[0] = phase_a_compute(hf0, chunks[0][3])
            wv_f = setup_dma_b()
            setup_compute_b(wv_f)
            hf1 = phase_a_dma(*chunks[1])
            abuf[1] = phase_a_compute(hf1, chunks[1][3])
            for k in range(NK):
                phase_b(*chunks[k], *abuf.pop(k), final=(k == NK - 1))
                if k + 2 < NK:
                    hfk = phase_a_dma(*chunks[k + 2])
                    abuf[k + 2] = phase_a_compute(hfk, chunks[k + 2][3])

    nc.compile()
    return nc


_NC_CACHE = None


def _get_nc():
    global _NC_CACHE
    if _NC_CACHE is None:
        _NC_CACHE = _build_program()
    return _NC_CACHE


def kernel(**inputs) -> np.ndarray:
    h_prime = np.ascontiguousarray(inputs["h_prime"], dtype=np.float32)
    h_llm = np.ascontiguousarray(inputs["h_llm"], dtype=np.float32)
    wnames = ["Wq", "Wk", "Wv", "Wg", "Wb", "We", "bq", "bk", "bv", "bg", "bb"]
    wmap = {n: np.ascontiguousarray(inputs[n], dtype=np.float32) for n in wnames}

    nc = _get_nc()
    in_maps = []
    for b in range(B):
        m = {"h": h_prime[b], "hl": h_llm[b]}
        m.update(wmap)
        in_maps.append(m)

    res = run_bass_kernel_spmd(nc, in_maps, list(range(B)), trace=TRACE,
                               **(RUN_KWARGS or {}))
    global LAST_EXEC_TIME_NS, LAST_RESULT
    LAST_EXEC_TIME_NS = res.exec_time_ns
    LAST_RESULT = res
    out = np.stack([res.results[b]["out"] for b in range(B)], axis=0)
    return out.astype(np.float32)


LAST_EXEC_TIME_NS = None
LAST_RESULT = None
TRACE = False
RUN_KWARGS = None


if __name__ == "__main__":
    nc = _get_nc()
    print("program built ok")
